# revision 1
# baseline (speedup 1.0000x reference)
"""GRU-cell-variant kernel for Trainium2, data-parallel over batch on 8 cores.

Reference (per batch row b, hidden size H=1024):
    gates = sigmoid(x @ W_ih + b_ih + h @ W_hh + b_hh)   # [B, 2H]
    z, r  = gates[:, :H], gates[:, H:]
    cand  = tanh(x @ W_c + b_c + r * (h @ W_hc + b_hc))
    out   = (1 - z) * h + z * cand

Design:
  - 8-way batch shard (1024 rows/core), weights replicated. No collectives.
  - Everything on-chip is computed TRANSPOSED: out.T[o, b]. That way weight
    tiles [k, o] load naturally as the stationary operand, host-pre-transposed
    x.T / h.T serve as the moving operand, and all biases are per-partition
    (free bias-add on the ACT engine).
  - Matmuls in fp16 (1 cycle/row on the PE) with fp32 PSUM accumulation;
    elementwise math and h-residual in fp32.
  - Host packs weights/activations into the exact SBUF layouts so every DMA
    is a dense 2D copy with >=2KB per-partition lines.
"""

import numpy as np

import concourse.bass as bass
import concourse.mybir as mybir
import concourse.tile as tile
from concourse import bacc
from concourse.bass_utils import run_bass_kernel_spmd

N_CORES = 8
B = 8192
H = 1024
BL = B // N_CORES  # batch rows per core
P = 128
KC = H // P  # 8 contraction chunks of 128 per 1024-wide operand
NJ = H // P  # 8 hidden-dim tiles
NB = BL // 512  # 2 moving halves of 512 batch columns

F16 = mybir.dt.float16
F32 = mybir.dt.float32
AF = mybir.ActivationFunctionType
ALU = mybir.AluOpType

_CACHE = {}


def _build_program():
    nc = bacc.Bacc(
        "TRN2",
        target_bir_lowering=False,
        debug=False,
        enable_asserts=False,
        num_devices=N_CORES,
    )

    # DRAM inputs, already packed on the host into SBUF-friendly layouts.
    # xT/hT:  [p, kc*BL + b]        = x[b, kc*128 + p]           (fp16)
    # hT32:   same layout, fp32 (residual path)
    # Wg:     [p, t*2048 + kc*128 + jj] = Wg_full[kc*128+p, t*128+jj]  (fp16)
    #          t in [0,16): gate output tile; kc in [0,16): contraction over [x;h]
    # Wc/Whc: [p, j*1024 + kc*128 + jj] = W[kc*128+p, j*128+jj]  (fp16)
    # bg:     [p, t] = (b_ih+b_hh)[t*128+p]; bc/bhc analogous.
    xT = nc.dram_tensor("xT", [P, KC * BL], F16, kind="ExternalInput").ap()
    hT = nc.dram_tensor("hT", [P, KC * BL], F16, kind="ExternalInput").ap()
    hT32 = nc.dram_tensor("hT32", [P, NJ * BL], F32, kind="ExternalInput").ap()
    Wg = nc.dram_tensor("Wg", [P, 16 * 2048], F16, kind="ExternalInput").ap()
    Wc = nc.dram_tensor("Wc", [P, NJ * H], F16, kind="ExternalInput").ap()
    Whc = nc.dram_tensor("Whc", [P, NJ * H], F16, kind="ExternalInput").ap()
    bg = nc.dram_tensor("bg", [P, 16], F32, kind="ExternalInput").ap()
    bc = nc.dram_tensor("bc", [P, NJ], F32, kind="ExternalInput").ap()
    bhc = nc.dram_tensor("bhc", [P, NJ], F32, kind="ExternalInput").ap()
    outT = nc.dram_tensor("outT", [P, NJ * BL], F32, kind="ExternalOutput").ap()

    with tile.TileContext(nc) as tc:
        with (
            tc.tile_pool(name="const", bufs=1) as cpool,
            tc.tile_pool(name="wg", bufs=4) as wgpool,
            tc.tile_pool(name="wsm", bufs=4) as wsmpool,
            tc.tile_pool(name="psum", bufs=8, space="PSUM") as ppool,
            tc.tile_pool(name="gates", bufs=6) as gpool,
            tc.tile_pool(name="work", bufs=10) as wpool,
        ):
            xT_sb = cpool.tile([P, KC * BL], F16, tag="xT")
            nc.sync.dma_start(xT_sb[:], xT[:])
            hT_sb = cpool.tile([P, KC * BL], F16, tag="hT")
            nc.sync.dma_start(hT_sb[:], hT[:])
            hT32_sb = cpool.tile([P, NJ * BL], F32, tag="hT32")
            nc.sync.dma_start(hT32_sb[:], hT32[:])
            bg_sb = cpool.tile([P, 16], F32, tag="bg")
            nc.sync.dma_start(bg_sb[:], bg[:])
            bc_sb = cpool.tile([P, NJ], F32, tag="bc")
            nc.sync.dma_start(bc_sb[:], bc[:])
            bhc_sb = cpool.tile([P, NJ], F32, tag="bhc")
            nc.sync.dma_start(bhc_sb[:], bhc[:])

            def gate_matmuls(psum, w_sb, b0):
                # accumulate over [x;h]: kc<8 reads xT, kc>=8 reads hT
                for kc in range(2 * KC):
                    src = xT_sb if kc < KC else hT_sb
                    off = (kc % KC) * BL + b0
                    nc.tensor.matmul(
                        psum[:],
                        lhsT=w_sb[:, kc * P : (kc + 1) * P],
                        rhs=src[:, off : off + 512],
                        start=(kc == 0),
                        stop=(kc == 2 * KC - 1),
                    )

            def cand_matmuls(psum, w_sb, src, b0):
                for kc in range(KC):
                    off = kc * BL + b0
                    nc.tensor.matmul(
                        psum[:],
                        lhsT=w_sb[:, kc * P : (kc + 1) * P],
                        rhs=src[:, off : off + 512],
                        start=(kc == 0),
                        stop=(kc == KC - 1),
                    )

            for j in range(NJ):
                wz = wgpool.tile([P, 2048], F16, tag="wg")
                nc.sync.dma_start(wz[:], Wg[:, j * 2048 : (j + 1) * 2048])
                wr = wgpool.tile([P, 2048], F16, tag="wg")
                nc.sync.dma_start(wr[:], Wg[:, (NJ + j) * 2048 : (NJ + j + 1) * 2048])
                whc_w = wsmpool.tile([P, H], F16, tag="wsm")
                nc.sync.dma_start(whc_w[:], Whc[:, j * H : (j + 1) * H])
                wc_w = wsmpool.tile([P, H], F16, tag="wsm")
                nc.sync.dma_start(wc_w[:], Wc[:, j * H : (j + 1) * H])

                for b in range(NB):
                    b0 = b * 512
                    hoff = j * BL + b0  # slice of hidden tile j in [p, j*BL+b] layout

                    pz = ppool.tile([P, 512], F32, tag="ps")
                    gate_matmuls(pz, wz, b0)
                    z_sb = gpool.tile([P, 512], F32, tag="g")
                    nc.scalar.activation(z_sb[:], pz[:], AF.Sigmoid, bias=bg_sb[:, j : j + 1])

                    pr = ppool.tile([P, 512], F32, tag="ps")
                    gate_matmuls(pr, wr, b0)
                    r_sb = gpool.tile([P, 512], F32, tag="g")
                    nc.scalar.activation(
                        r_sb[:], pr[:], AF.Sigmoid, bias=bg_sb[:, NJ + j : NJ + j + 1]
                    )

                    ph = ppool.tile([P, 512], F32, tag="ps")
                    cand_matmuls(ph, whc_w, hT_sb, b0)
                    px = ppool.tile([P, 512], F32, tag="ps")
                    cand_matmuls(px, wc_w, xT_sb, b0)

                    # rh = (hc + b_hc) * r   (one DVE op)
                    rh = wpool.tile([P, 512], F32, tag="w")
                    nc.vector.scalar_tensor_tensor(
                        rh[:], ph[:], bhc_sb[:, j : j + 1], r_sb[:], ALU.add, ALU.mult
                    )
                    s = wpool.tile([P, 512], F32, tag="w")
                    nc.vector.tensor_add(s[:], px[:], rh[:])
                    cand = wpool.tile([P, 512], F32, tag="w")
                    nc.scalar.activation(cand[:], s[:], AF.Tanh, bias=bc_sb[:, j : j + 1])

                    # out = h + z * (cand - h)
                    d = wpool.tile([P, 512], F32, tag="w")
                    nc.vector.tensor_sub(d[:], cand[:], hT32_sb[:, hoff : hoff + 512])
                    m = wpool.tile([P, 512], F32, tag="w")
                    nc.vector.tensor_mul(m[:], z_sb[:], d[:])
                    o_sb = wpool.tile([P, 512], F32, tag="w")
                    nc.vector.tensor_add(o_sb[:], m[:], hT32_sb[:, hoff : hoff + 512])
                    nc.sync.dma_start(outT[:, hoff : hoff + 512], o_sb[:])

    nc.compile()
    return nc


def _pack_weights(W_ih, b_ih, W_hh, b_hh, W_c, b_c, W_hc, b_hc):
    f16 = np.float16
    Wg_full = np.concatenate([W_ih, W_hh], axis=0)  # [2H, 2H] = [k, o]
    WgH = np.ascontiguousarray(
        Wg_full.reshape(16, P, 16, P).transpose(1, 2, 0, 3).reshape(P, 16 * 2048)
    ).astype(f16)
    WcH = np.ascontiguousarray(
        W_c.reshape(KC, P, NJ, P).transpose(1, 2, 0, 3).reshape(P, NJ * H)
    ).astype(f16)
    WhcH = np.ascontiguousarray(
        W_hc.reshape(KC, P, NJ, P).transpose(1, 2, 0, 3).reshape(P, NJ * H)
    ).astype(f16)
    bgH = np.ascontiguousarray((b_ih + b_hh).reshape(16, P).T).astype(np.float32)
    bcH = np.ascontiguousarray(b_c.reshape(NJ, P).T).astype(np.float32)
    bhcH = np.ascontiguousarray(b_hc.reshape(NJ, P).T).astype(np.float32)
    return WgH, WcH, WhcH, bgH, bcH, bhcH


def _pack_acts(a, dtype):
    # [BL, H] -> [p, kc*BL + b] with a[b, kc*128+p]
    return np.ascontiguousarray(
        a.T.reshape(KC, P, BL).transpose(1, 0, 2).reshape(P, KC * BL)
    ).astype(dtype)


def kernel(input, hx, W_ih, b_ih, W_hh, b_hh, W_c, b_c, W_hc, b_hc):
    input = np.asarray(input, np.float32)
    hx = np.asarray(hx, np.float32)
    if "nc" not in _CACHE:
        _CACHE["nc"] = _build_program()
    nc = _CACHE["nc"]

    WgH, WcH, WhcH, bgH, bcH, bhcH = _pack_weights(
        np.asarray(W_ih, np.float32), np.asarray(b_ih, np.float32),
        np.asarray(W_hh, np.float32), np.asarray(b_hh, np.float32),
        np.asarray(W_c, np.float32), np.asarray(b_c, np.float32),
        np.asarray(W_hc, np.float32), np.asarray(b_hc, np.float32),
    )

    in_maps = []
    for i in range(N_CORES):
        xs = input[i * BL : (i + 1) * BL]
        hs = hx[i * BL : (i + 1) * BL]
        in_maps.append(
            {
                "xT": _pack_acts(xs, np.float16),
                "hT": _pack_acts(hs, np.float16),
                "hT32": _pack_acts(hs, np.float32),
                "Wg": WgH,
                "Wc": WcH,
                "Whc": WhcH,
                "bg": bgH,
                "bc": bcH,
                "bhc": bhcH,
            }
        )

    res = run_bass_kernel_spmd(nc, in_maps, core_ids=list(range(N_CORES)))
    out = np.empty((B, H), np.float32)
    for i, r in enumerate(res.results):
        o = r["outT"].reshape(P, NJ, BL).transpose(2, 1, 0).reshape(BL, H)
        out[i * BL : (i + 1) * BL] = o
    return out


# revision 3
# speedup vs baseline: 1.0756x; 1.0756x over previous
"""GRU-cell-variant kernel for Trainium2, data-parallel over batch on 8 cores.

Reference (per batch row b, hidden size H=1024):
    gates = sigmoid(x @ W_ih + b_ih + h @ W_hh + b_hh)   # [B, 2H]
    z, r  = gates[:, :H], gates[:, H:]
    cand  = tanh(x @ W_c + b_c + r * (h @ W_hc + b_hc))
    out   = (1 - z) * h + z * cand

Design:
  - 8-way batch shard (1024 rows/core), weights replicated. No collectives.
  - Everything on-chip is computed TRANSPOSED: out.T[o, b]. That way weight
    tiles [k, o] load naturally as the stationary operand, host-pre-transposed
    x.T / h.T serve as the moving operand, and all biases are per-partition
    (free bias-add on the ACT engine).
  - Matmuls in fp16 (1 cycle/row on the PE) with fp32 PSUM accumulation;
    elementwise math and h-residual in fp32.
  - Host packs weights/activations into the exact SBUF layouts so every DMA
    is a dense 2D copy with >=2KB per-partition lines.
"""

import numpy as np

import concourse.bass as bass
import concourse.mybir as mybir
import concourse.tile as tile
from concourse import bacc
from concourse.bass_utils import run_bass_kernel_spmd

N_CORES = 8
B = 8192
H = 1024
BL = B // N_CORES  # batch rows per core
P = 128
KC = H // P  # 8 contraction chunks of 128 per 1024-wide operand
NJ = H // P  # 8 hidden-dim tiles
NB = BL // 512  # 2 moving halves of 512 batch columns

F16 = mybir.dt.float16
F32 = mybir.dt.float32
AF = mybir.ActivationFunctionType
ALU = mybir.AluOpType

_CACHE = {}


def _build_program():
    nc = bacc.Bacc(
        "TRN2",
        target_bir_lowering=False,
        debug=False,
        enable_asserts=False,
        num_devices=N_CORES,
    )

    # DRAM inputs, already packed on the host into SBUF-friendly layouts.
    # xT/hT:  [p, kc*BL + b]        = x[b, kc*128 + p]           (fp16)
    # hT32:   same layout, fp32 (residual path)
    # Wg:     [p, t*2048 + kc*128 + jj] = Wg_full[kc*128+p, t*128+jj]  (fp16)
    #          t in [0,16): gate output tile; kc in [0,16): contraction over [x;h]
    # Wc/Whc: [p, j*1024 + kc*128 + jj] = W[kc*128+p, j*128+jj]  (fp16)
    # bg:     [p, t] = (b_ih+b_hh)[t*128+p]; bc/bhc analogous.
    xT = nc.dram_tensor("xT", [P, KC * BL], F16, kind="ExternalInput").ap()
    hT = nc.dram_tensor("hT", [P, KC * BL], F16, kind="ExternalInput").ap()
    hT32 = nc.dram_tensor("hT32", [P, NJ * BL], F32, kind="ExternalInput").ap()
    Wg = nc.dram_tensor("Wg", [P, 16 * 2048], F16, kind="ExternalInput").ap()
    Wc = nc.dram_tensor("Wc", [P, NJ * H], F16, kind="ExternalInput").ap()
    Whc = nc.dram_tensor("Whc", [P, NJ * H], F16, kind="ExternalInput").ap()
    bg = nc.dram_tensor("bg", [P, 16], F32, kind="ExternalInput").ap()
    bc = nc.dram_tensor("bc", [P, NJ], F32, kind="ExternalInput").ap()
    bhc = nc.dram_tensor("bhc", [P, NJ], F32, kind="ExternalInput").ap()
    outT = nc.dram_tensor("outT", [P, NJ * BL], F32, kind="ExternalOutput").ap()

    with tile.TileContext(nc) as tc:
        with (
            tc.tile_pool(name="const", bufs=1) as cpool,
            tc.tile_pool(name="wg", bufs=4) as wgpool,
            tc.tile_pool(name="wsm", bufs=4) as wsmpool,
            tc.tile_pool(name="psum", bufs=8, space="PSUM") as ppool,
            tc.tile_pool(name="gates", bufs=6) as gpool,
            tc.tile_pool(name="work", bufs=10) as wpool,
        ):
            # Small constants first (they gate the ACT ops).
            bg_sb = cpool.tile([P, 16], F32, tag="bg")
            nc.sync.dma_start(bg_sb[:], bg[:])
            bc_sb = cpool.tile([P, NJ], F32, tag="bc")
            nc.sync.dma_start(bc_sb[:], bc[:])
            bhc_sb = cpool.tile([P, NJ], F32, tag="bhc")
            nc.sync.dma_start(bhc_sb[:], bhc[:])

            # Resident activations, loaded in per-kc chunks so the first
            # matmuls only wait on the first 128KB-256KB of traffic instead
            # of the full 8MB input preamble. hT32 (residual path, fp32) is
            # streamed per-j inside the loop — it isn't needed until the
            # first elementwise stage.
            xT_sb = cpool.tile([P, KC * BL], F16, tag="xT")
            hT_sb = cpool.tile([P, KC * BL], F16, tag="hT")
            hT32_sb = cpool.tile([P, NJ * BL], F32, tag="hT32")

            def gate_matmuls(psum, w_sb, b0):
                # accumulate over [x;h]: kc<8 reads xT, kc>=8 reads hT
                for kc in range(2 * KC):
                    src = xT_sb if kc < KC else hT_sb
                    off = (kc % KC) * BL + b0
                    nc.tensor.matmul(
                        psum[:],
                        lhsT=w_sb[:, kc * P : (kc + 1) * P],
                        rhs=src[:, off : off + 512],
                        start=(kc == 0),
                        stop=(kc == 2 * KC - 1),
                    )

            def cand_matmuls(psum, w_sb, src, b0):
                for kc in range(KC):
                    off = kc * BL + b0
                    nc.tensor.matmul(
                        psum[:],
                        lhsT=w_sb[:, kc * P : (kc + 1) * P],
                        rhs=src[:, off : off + 512],
                        start=(kc == 0),
                        stop=(kc == KC - 1),
                    )

            def load_wg(dst, t):
                # 4 chunks of [128, 512] so matmuls unblock per 4-kc group
                for c in range(4):
                    nc.sync.dma_start(
                        dst[:, c * 512 : (c + 1) * 512],
                        Wg[:, t * 2048 + c * 512 : t * 2048 + (c + 1) * 512],
                    )

            def load_wsm(dst, src, j):
                for c in range(2):
                    nc.sync.dma_start(
                        dst[:, c * 512 : (c + 1) * 512],
                        src[:, j * H + c * 512 : j * H + (c + 1) * 512],
                    )

            for j in range(NJ):
                wz = wgpool.tile([P, 2048], F16, tag="wg")
                if j == 0:
                    # Critical feed order for the cold start: first z-weight
                    # chunk, then x chunks, then h chunks, then the rest.
                    load_wg(wz, 0)
                    for kc in range(KC):
                        nc.sync.dma_start(
                            xT_sb[:, kc * BL : (kc + 1) * BL],
                            xT[:, kc * BL : (kc + 1) * BL],
                        )
                    for kc in range(KC):
                        nc.sync.dma_start(
                            hT_sb[:, kc * BL : (kc + 1) * BL],
                            hT[:, kc * BL : (kc + 1) * BL],
                        )
                else:
                    load_wg(wz, j)
                wr = wgpool.tile([P, 2048], F16, tag="wg")
                load_wg(wr, NJ + j)
                whc_w = wsmpool.tile([P, H], F16, tag="wsm")
                load_wsm(whc_w, Whc, j)
                wc_w = wsmpool.tile([P, H], F16, tag="wsm")
                load_wsm(wc_w, Wc, j)
                nc.sync.dma_start(
                    hT32_sb[:, j * BL : (j + 1) * BL], hT32[:, j * BL : (j + 1) * BL]
                )

                for b in range(NB):
                    b0 = b * 512
                    hoff = j * BL + b0  # slice of hidden tile j in [p, j*BL+b] layout

                    pz = ppool.tile([P, 512], F32, tag="ps")
                    gate_matmuls(pz, wz, b0)
                    z_sb = gpool.tile([P, 512], F32, tag="g")
                    nc.scalar.activation(z_sb[:], pz[:], AF.Sigmoid, bias=bg_sb[:, j : j + 1])

                    pr = ppool.tile([P, 512], F32, tag="ps")
                    gate_matmuls(pr, wr, b0)
                    r_sb = gpool.tile([P, 512], F32, tag="g")
                    nc.scalar.activation(
                        r_sb[:], pr[:], AF.Sigmoid, bias=bg_sb[:, NJ + j : NJ + j + 1]
                    )

                    ph = ppool.tile([P, 512], F32, tag="ps")
                    cand_matmuls(ph, whc_w, hT_sb, b0)
                    px = ppool.tile([P, 512], F32, tag="ps")
                    cand_matmuls(px, wc_w, xT_sb, b0)

                    # rh = (hc + b_hc) * r   (one DVE op)
                    rh = wpool.tile([P, 512], F32, tag="w")
                    nc.vector.scalar_tensor_tensor(
                        rh[:], ph[:], bhc_sb[:, j : j + 1], r_sb[:], ALU.add, ALU.mult
                    )
                    s = wpool.tile([P, 512], F32, tag="w")
                    nc.vector.tensor_add(s[:], px[:], rh[:])
                    cand = wpool.tile([P, 512], F32, tag="w")
                    nc.scalar.activation(cand[:], s[:], AF.Tanh, bias=bc_sb[:, j : j + 1])

                    # out = h + z * (cand - h)
                    d = wpool.tile([P, 512], F32, tag="w")
                    nc.vector.tensor_sub(d[:], cand[:], hT32_sb[:, hoff : hoff + 512])
                    m = wpool.tile([P, 512], F32, tag="w")
                    nc.vector.tensor_mul(m[:], z_sb[:], d[:])
                    o_sb = wpool.tile([P, 512], F32, tag="w")
                    nc.vector.tensor_add(o_sb[:], m[:], hT32_sb[:, hoff : hoff + 512])
                    nc.sync.dma_start(outT[:, hoff : hoff + 512], o_sb[:])

    nc.compile()
    return nc


def _pack_weights(W_ih, b_ih, W_hh, b_hh, W_c, b_c, W_hc, b_hc):
    f16 = np.float16
    Wg_full = np.concatenate([W_ih, W_hh], axis=0)  # [2H, 2H] = [k, o]
    WgH = np.ascontiguousarray(
        Wg_full.reshape(16, P, 16, P).transpose(1, 2, 0, 3).reshape(P, 16 * 2048)
    ).astype(f16)
    WcH = np.ascontiguousarray(
        W_c.reshape(KC, P, NJ, P).transpose(1, 2, 0, 3).reshape(P, NJ * H)
    ).astype(f16)
    WhcH = np.ascontiguousarray(
        W_hc.reshape(KC, P, NJ, P).transpose(1, 2, 0, 3).reshape(P, NJ * H)
    ).astype(f16)
    bgH = np.ascontiguousarray((b_ih + b_hh).reshape(16, P).T).astype(np.float32)
    bcH = np.ascontiguousarray(b_c.reshape(NJ, P).T).astype(np.float32)
    bhcH = np.ascontiguousarray(b_hc.reshape(NJ, P).T).astype(np.float32)
    return WgH, WcH, WhcH, bgH, bcH, bhcH


def _pack_acts(a, dtype):
    # [BL, H] -> [p, kc*BL + b] with a[b, kc*128+p]
    return np.ascontiguousarray(
        a.T.reshape(KC, P, BL).transpose(1, 0, 2).reshape(P, KC * BL)
    ).astype(dtype)


def kernel(input, hx, W_ih, b_ih, W_hh, b_hh, W_c, b_c, W_hc, b_hc):
    input = np.asarray(input, np.float32)
    hx = np.asarray(hx, np.float32)
    if "nc" not in _CACHE:
        _CACHE["nc"] = _build_program()
    nc = _CACHE["nc"]

    WgH, WcH, WhcH, bgH, bcH, bhcH = _pack_weights(
        np.asarray(W_ih, np.float32), np.asarray(b_ih, np.float32),
        np.asarray(W_hh, np.float32), np.asarray(b_hh, np.float32),
        np.asarray(W_c, np.float32), np.asarray(b_c, np.float32),
        np.asarray(W_hc, np.float32), np.asarray(b_hc, np.float32),
    )

    in_maps = []
    for i in range(N_CORES):
        xs = input[i * BL : (i + 1) * BL]
        hs = hx[i * BL : (i + 1) * BL]
        in_maps.append(
            {
                "xT": _pack_acts(xs, np.float16),
                "hT": _pack_acts(hs, np.float16),
                "hT32": _pack_acts(hs, np.float32),
                "Wg": WgH,
                "Wc": WcH,
                "Whc": WhcH,
                "bg": bgH,
                "bc": bcH,
                "bhc": bhcH,
            }
        )

    res = run_bass_kernel_spmd(nc, in_maps, core_ids=list(range(N_CORES)))
    out = np.empty((B, H), np.float32)
    for i, r in enumerate(res.results):
        o = r["outT"].reshape(P, NJ, BL).transpose(2, 1, 0).reshape(BL, H)
        out[i * BL : (i + 1) * BL] = o
    return out
